# revision 2
# baseline (speedup 1.0000x reference)
"""Trainium2 Bass kernel for the stacked-LSTMCell network.

Semantics recap (derived from the reference):
  - enc [T=256, B=2048, 136] is flat-reinterpreted so that the scan runs
    2048 independent sequences of length 256.  Flat row r = t*2048 + b
    belongs to sequence s' = r // 256 at step j = r % 256.
  - Sharding: core m owns t in [32m, 32m+32) (=> sequences [256m, 256m+256)).
  - Per core: 256 sequences, 256 steps, feature-major tiles [feat, batch].

Tricks baked into host-side weight preprocessing:
  - tanh(x) = 2*sigmoid(2x) - 1: all gate activations become Sigmoid (one
    ACT table set, one big sigmoid per gate block); the *2 on g-gate inputs
    is folded into the weights, the affine fixups into fused DVE
    scalar_tensor_tensor ops.
  - The h-states are stored halved (H = h/2, exactly absorbing the 2s-1
    fixup); consumers' weights are pre-doubled.
  - LSTM1 bias rides an extra all-ones row of X; LSTM2 bias rides an
    all-ones row appended to the H2 state tile.
"""

import sys

sys.path.insert(0, "/opt/trn_rl_repo")

import numpy as np
import ml_dtypes

import concourse.bass as bass
import concourse.tile as tile
import concourse.mybir as mybir
from concourse import bacc
from concourse.bass_utils import run_bass_kernel_spmd

BF16 = mybir.dt.bfloat16
F32 = mybir.dt.float32
AF = mybir.ActivationFunctionType
OP = mybir.AluOpType
NPBF16 = ml_dtypes.bfloat16

T_FULL, B_FULL = 256, 2048
N_CORES = 8
SEQ = 256            # sequences per core
H1, H2 = 128, 32
EMB_W = [32, 16, 16, 8, 8, 8]


# ---------------------------------------------------------------- weights

def prep_weights(W_ih1, b_ih1, W_hh1, b_hh1, W_ih2, b_ih2, W_hh2, b_hh2,
                 W1, b1, W2, b2):
    """All the prescaling tricks, host side (cheap, <1MB of data)."""
    f32 = np.float32
    W_ih1, b_ih1 = np.asarray(W_ih1, f32), np.asarray(b_ih1, f32)
    W_hh1, b_hh1 = np.asarray(W_hh1, f32), np.asarray(b_hh1, f32)
    W_ih2, b_ih2 = np.asarray(W_ih2, f32), np.asarray(b_ih2, f32)
    W_hh2, b_hh2 = np.asarray(W_hh2, f32), np.asarray(b_hh2, f32)
    W1, b1 = np.asarray(W1, f32), np.asarray(b1, f32)
    W2, b2 = np.asarray(W2, f32), np.asarray(b2, f32)

    g = slice(256, 384)          # g-gate block in the 512-dim (i,f,g,o)
    # LSTM1: x-side weights augmented with a bias row (X row 136 == 1.0).
    wx = np.zeros((144, 512), f32)
    wx[0:136, :] = W_ih1.T
    wx[136, :] = b_ih1 + b_hh1
    wx[:, g] *= 2.0                          # g-gate: feed 2*pre (tanh trick)
    wx_hi = wx[0:128].astype(NPBF16)         # [128, 512]
    wx_lo = wx[128:144].astype(NPBF16)       # [16, 512]

    wh1 = (2.0 * W_hh1).T.copy()             # h1 stored halved
    wh1[:, g] *= 2.0
    wh1 = wh1.astype(NPBF16)                 # [128, 512]

    g2 = slice(64, 96)
    wih2 = (2.0 * W_ih2).T.copy()            # [128, 128]
    wih2[:, g2] *= 2.0
    wih2 = wih2.astype(NPBF16)

    whh2 = np.zeros((33, 128), f32)
    whh2[0:32] = (2.0 * W_hh2).T             # h2 stored halved
    whh2[32] = b_ih2 + b_hh2                 # bias row (H2 row 32 == 1.0)
    whh2[:, g2] *= 2.0
    whh2 = whh2.astype(NPBF16)               # [33, 128]

    w1rep = np.zeros((128, 32), f32)         # 4 stacked copies of (2*W1)^T
    w2blk = np.zeros((128, 4), f32)
    b1rep = np.zeros((128, 1), f32)
    for blk in range(4):
        w1rep[32 * blk:32 * blk + 32, 0:16] = (2.0 * W1).T
        w2blk[32 * blk:32 * blk + 16, blk] = W2[0]
        b1rep[32 * blk:32 * blk + 16, 0] = b1
    return dict(
        wx_hi=wx_hi, wx_lo=wx_lo, wh1=wh1, wih2=wih2, whh2=whh2,
        w1rep=w1rep.astype(NPBF16), w2blk=w2blk.astype(NPBF16),
        b1rep=b1rep, b2=float(b2[0]),
    )


# ------------------------------------------------------------- host X^T

def host_build_xt(mlp_tmpl_c, mlp_static_c, idx_c, tables_f32):
    """Per-core X^T (feature-major, (j,s)-column-ordered), bf16.

    mlp_tmpl_c [32,2048,32], mlp_static_c [32,2048,16], idx_c [32,2048,6].
    Returns xt_hi [128, 65536], xt_lo [16, 65536].
    """
    rows = mlp_tmpl_c.shape[0] * 2048
    X = np.zeros((rows, 144), np.float32)
    X[:, 0:32] = mlp_tmpl_c.reshape(rows, 32)
    X[:, 32:48] = mlp_static_c.reshape(rows, 16)
    idx = idx_c.reshape(rows, 6)
    off = 48
    for c, tbl in enumerate(tables_f32):
        w = tbl.shape[1]
        X[:, off:off + w] = tbl[idx[:, c]]
        off += w
    X[:, 136] = 1.0
    # rows r = 256*s + j  ->  cols c = j*256 + s
    Xp = X.reshape(256, 256, 144).transpose(1, 0, 2).reshape(rows, 144)
    XpT = np.ascontiguousarray(Xp.T).astype(NPBF16)
    return XpT[0:128], XpT[128:144]


# ------------------------------------------------------------ the program

def build_program(J=256, G=2, CHUNK=16):
    """Build + finalize the per-core Bass program.  J = steps to run."""
    SG = SEQ // G
    nc = bacc.Bacc()

    xt_hi = nc.declare_dram_parameter("xt_hi", [128, SEQ * 256], BF16, isOutput=False)
    xt_lo = nc.declare_dram_parameter("xt_lo", [16, SEQ * 256], BF16, isOutput=False)
    wxh = nc.declare_dram_parameter("wx_hi", [128, 512], BF16, isOutput=False)
    wxl = nc.declare_dram_parameter("wx_lo", [16, 512], BF16, isOutput=False)
    wh1d = nc.declare_dram_parameter("wh1", [128, 512], BF16, isOutput=False)
    wih2d = nc.declare_dram_parameter("wih2", [128, 128], BF16, isOutput=False)
    whh2d = nc.declare_dram_parameter("whh2", [33, 128], BF16, isOutput=False)
    w1d = nc.declare_dram_parameter("w1rep", [128, 32], BF16, isOutput=False)
    w2d = nc.declare_dram_parameter("w2blk", [128, 4], BF16, isOutput=False)
    b1d = nc.declare_dram_parameter("b1rep", [128, 1], F32, isOutput=False)
    b2_host = nc.declare_dram_parameter("b2h", [128, 1], F32, isOutput=False)
    out_d = nc.declare_dram_parameter("out", [256, 256], F32, isOutput=True)

    h2_dram = nc.dram_tensor("h2buf", [256, 32, 256], BF16)

    assert J % 4 == 0
    n_chunks = (J + CHUNK - 1) // CHUNK

    with tile.TileContext(nc) as tc:
        with (
            tc.tile_pool(name="wpool", bufs=1) as wpool,
            tc.tile_pool(name="state", bufs=1) as state,
            tc.tile_pool(name="xstage", bufs=2) as xstage,
            tc.tile_pool(name="work1", bufs=3) as work1,
            tc.tile_pool(name="work2", bufs=2) as work2,
            tc.tile_pool(name="g1p", bufs=3, space="PSUM") as g1p,
            tc.tile_pool(name="g2p", bufs=2, space="PSUM") as g2p,
        ):
            # ---- load weights
            wx_hi_t = wpool.tile([128, 512], BF16)
            nc.sync.dma_start(wx_hi_t[:], wxh[:])
            wx_lo_t = wpool.tile([16, 512], BF16)
            nc.sync.dma_start(wx_lo_t[:], wxl[:])
            wh1_t = wpool.tile([128, 512], BF16)
            nc.sync.dma_start(wh1_t[:], wh1d[:])
            wih2_t = wpool.tile([128, 128], BF16)
            nc.sync.dma_start(wih2_t[:], wih2d[:])
            whh2_t = wpool.tile([33, 128], BF16)
            nc.sync.dma_start(whh2_t[:], whh2d[:])
            # ---- persistent state
            c1s = []
            for gi in range(G):
                c1 = state.tile([128, SG], F32, tag=f"c1_{gi}")
                nc.vector.memset(c1[:], 0.0)
                c1s.append(c1)
            H1s = []
            for k in range(2):
                t = state.tile([128, 256], BF16, tag=f"H1_{k}")
                nc.vector.memset(t[:], 0.0)
                H1s.append(t)
            H2s = []
            for k in range(2):
                t = state.tile([33, 256], BF16, tag=f"H2_{k}")
                nc.vector.memset(t[0:32, :], 0.0)
                nc.vector.memset(t[32:33, :], 1.0)
                H2s.append(t)
            c2 = state.tile([32, 256], F32, tag="c2")
            nc.vector.memset(c2[:], 0.0)

            # ---- recurrent loop
            for ch in range(n_chunks):
                cols = min(CHUNK, J - ch * CHUNK) * 256
                xhi = xstage.tile([128, CHUNK * 256], BF16, tag="xhi")
                nc.sync.dma_start(
                    xhi[:, 0:cols],
                    xt_hi[:, ch * CHUNK * 256: ch * CHUNK * 256 + cols])
                xlo = xstage.tile([16, CHUNK * 256], BF16, tag="xlo")
                nc.sync.dma_start(
                    xlo[:, 0:cols],
                    xt_lo[:, ch * CHUNK * 256: ch * CHUNK * 256 + cols])

                for jj in range(min(CHUNK, J - ch * CHUNK)):
                    j = ch * CHUNK + jj
                    H1prev, H1cur = H1s[j % 2], H1s[(j + 1) % 2]
                    H2prev, H2cur = H2s[j % 2], H2s[(j + 1) % 2]
                    # ---------------- LSTM1 (G groups over the 256 seqs)
                    for gi in range(G):
                        xc = jj * 256 + gi * SG
                        g1 = g1p.tile([128, 4 * SG], F32, tag="g1")
                        for gate in range(4):
                            o = g1[:, gate * SG:(gate + 1) * SG]
                            nc.tensor.matmul(
                                o, wx_hi_t[:, gate * 128:(gate + 1) * 128],
                                xhi[:, xc:xc + SG], start=True, stop=False)
                            nc.tensor.matmul(
                                o, wx_lo_t[:, gate * 128:(gate + 1) * 128],
                                xlo[:, xc:xc + SG], start=False, stop=False)
                            nc.tensor.matmul(
                                o, wh1_t[:, gate * 128:(gate + 1) * 128],
                                H1prev[:, gi * SG:(gi + 1) * SG],
                                start=False, stop=True)
                        S1 = work1.tile([128, 4 * SG], BF16, tag="S1")
                        nc.scalar.activation(S1[:], g1[:], AF.Sigmoid)
                        u = work1.tile([128, SG], BF16, tag="u")
                        nc.vector.scalar_tensor_tensor(
                            u[:], S1[:, 2 * SG:3 * SG], 0.5, S1[:, 0:SG],
                            OP.subtract, OP.mult)
                        v = work1.tile([128, SG], F32, tag="v")
                        nc.vector.tensor_tensor(
                            v[:], S1[:, SG:2 * SG], c1s[gi][:], op=OP.mult)
                        nc.vector.scalar_tensor_tensor(
                            c1s[gi][:], u[:], 2.0, v[:], OP.mult, OP.add)
                        Sc = work1.tile([128, SG], BF16, tag="Sc")
                        nc.scalar.activation(
                            Sc[:], c1s[gi][:], AF.Sigmoid, scale=2.0)
                        nc.vector.scalar_tensor_tensor(
                            H1cur[:, gi * SG:(gi + 1) * SG],
                            Sc[:], 0.5, S1[:, 3 * SG:4 * SG],
                            OP.subtract, OP.mult)
                    # ---------------- LSTM2 (whole 256 batch)
                    g2a = g2p.tile([32, 512], F32, tag="g2a")
                    g2b = g2p.tile([32, 512], F32, tag="g2b")
                    for gate in range(4):
                        tgt = g2a if gate < 2 else g2b
                        col = (gate % 2) * 256
                        o = tgt[:, col:col + 256]
                        nc.tensor.matmul(
                            o, wih2_t[:, 32 * gate:32 * gate + 32],
                            H1cur[:], start=True, stop=False)
                        nc.tensor.matmul(
                            o, whh2_t[:, 32 * gate:32 * gate + 32],
                            H2prev[:], start=False, stop=True)
                    S2a = work2.tile([32, 512], BF16, tag="S2a")
                    nc.scalar.activation(S2a[:], g2a[:], AF.Sigmoid)
                    S2b = work2.tile([32, 512], BF16, tag="S2b")
                    nc.scalar.activation(S2b[:], g2b[:], AF.Sigmoid)
                    u2 = work2.tile([32, 256], BF16, tag="u2")
                    nc.vector.scalar_tensor_tensor(
                        u2[:], S2b[:, 0:256], 0.5, S2a[:, 0:256],
                        OP.subtract, OP.mult)
                    v2 = work2.tile([32, 256], F32, tag="v2")
                    nc.vector.tensor_tensor(
                        v2[:], S2a[:, 256:512], c2[:], op=OP.mult)
                    nc.vector.scalar_tensor_tensor(
                        c2[:], u2[:], 2.0, v2[:], OP.mult, OP.add)
                    Sc2 = work2.tile([32, 256], BF16, tag="Sc2")
                    nc.scalar.activation(Sc2[:], c2[:], AF.Sigmoid, scale=2.0)
                    nc.vector.scalar_tensor_tensor(
                        H2cur[0:32, :], Sc2[:], 0.5, S2b[:, 256:512],
                        OP.subtract, OP.mult)
                    nc.sync.dma_start(h2_dram[j], H2cur[0:32, :])

        # ---------------- output head (no recurrence -> bulk)
        with (
            tc.tile_pool(name="owpool", bufs=1) as owpool,
            tc.tile_pool(name="opool", bufs=2) as opool,
            tc.tile_pool(name="olin", bufs=2, space="PSUM") as olin,
            tc.tile_pool(name="oout", bufs=2, space="PSUM") as oout,
        ):
            w1rep_t = owpool.tile([128, 32], BF16)
            nc.sync.dma_start(w1rep_t[:], w1d[:])
            w2blk_t = owpool.tile([128, 4], BF16)
            nc.sync.dma_start(w2blk_t[:], w2d[:])
            b1rep_t = owpool.tile([128, 1], F32)
            nc.sync.dma_start(b1rep_t[:], b1d[:])
            b2t2 = owpool.tile([128, 1], F32)
            nc.sync.dma_start(b2t2[:], b2_host[:])

            out_ps = None
            for q in range(J // 4):
                h2t = opool.tile([128, 256], BF16, tag="h2t")
                nc.sync.dma_start(h2t[:], h2_dram[4 * q:4 * q + 4])
                lin_ps = olin.tile([128, 256], F32, tag="lin")
                for blk in range(4):
                    nc.tensor.matmul(
                        lin_ps[32 * blk:32 * blk + 32, :],
                        w1rep_t[32 * blk:32 * blk + 32, :],
                        h2t[32 * blk:32 * blk + 32, :],
                        start=True, stop=True,
                        tile_position=(32 * blk, 32 * blk))
                lin_sb = opool.tile([128, 256], BF16, tag="linsb")
                nc.scalar.activation(
                    lin_sb[:], lin_ps[:], AF.Relu, bias=b1rep_t[:, 0:1])
                if q % 4 == 0:
                    out_ps = oout.tile([128, 256], F32, tag="outps")
                qi = q % 4
                nc.tensor.matmul(
                    out_ps[32 * qi:32 * qi + 4, :], w2blk_t[:], lin_sb[:],
                    start=True, stop=True, tile_position=(0, 32 * qi))
                if qi == 3 or q == J // 4 - 1:
                    out_sb = opool.tile([128, 256], F32, tag="outsb")
                    nc.scalar.activation(
                        out_sb[:], out_ps[:], AF.Sigmoid, bias=b2t2[:, 0:1])
                    Q = q // 4
                    for k in range(qi + 1):
                        nc.sync.dma_start(
                            out_d[16 * Q + 4 * k:16 * Q + 4 * k + 4, :],
                            out_sb[32 * k:32 * k + 4, :])

    nc.finalize()
    return nc


# ------------------------------------------------------------------ entry

_PROGRAM_CACHE = {}


def _get_program(J=256, G=2, CHUNK=16):
    key = (J, G, CHUNK)
    if key not in _PROGRAM_CACHE:
        _PROGRAM_CACHE[key] = build_program(J, G, CHUNK)
    return _PROGRAM_CACHE[key]


def make_in_maps(mlp_static_data, mlp_tmpl_data, emb_static_data,
                 emb_tmpl_data, emb_tables, weights):
    """Shard + host-prep the full inputs into 8 per-core input maps."""
    tables_f32 = [np.asarray(t, np.float32) for t in emb_tables]
    idx_full = np.concatenate(
        [np.asarray(emb_tmpl_data), np.asarray(emb_static_data)],
        axis=2).astype(np.int32)
    mlp_tmpl = np.asarray(mlp_tmpl_data, np.float32)
    mlp_static = np.asarray(mlp_static_data, np.float32)
    in_maps = []
    for m in range(N_CORES):
        sl = slice(32 * m, 32 * m + 32)
        xt_hi, xt_lo = host_build_xt(
            mlp_tmpl[sl], mlp_static[sl], idx_full[sl], tables_f32)
        in_maps.append(dict(
            xt_hi=xt_hi, xt_lo=xt_lo,
            wx_hi=weights["wx_hi"], wx_lo=weights["wx_lo"],
            wh1=weights["wh1"], wih2=weights["wih2"], whh2=weights["whh2"],
            w1rep=weights["w1rep"], w2blk=weights["w2blk"],
            b1rep=weights["b1rep"],
            b2h=np.full((128, 1), weights["b2"], np.float32),
        ))
    return in_maps


def assemble_output(results):
    outs = []
    for m in range(N_CORES):
        od = np.asarray(results[m]["out"], np.float32)   # [j, s]
        outs.append(od.T.reshape(32, 2048))              # r = 256 s + j
    return np.concatenate(outs, axis=0)


def kernel(mlp_static_data, mlp_tmpl_data, emb_static_data, emb_tmpl_data,
           emb_tables, W_ih1, b_ih1, W_hh1, b_hh1, W_ih2, b_ih2, W_hh2,
           b_hh2, W1, b1, W2, b2):
    weights = prep_weights(W_ih1, b_ih1, W_hh1, b_hh1, W_ih2, b_ih2,
                           W_hh2, b_hh2, W1, b1, W2, b2)
    in_maps = make_in_maps(mlp_static_data, mlp_tmpl_data, emb_static_data,
                           emb_tmpl_data, emb_tables, weights)
    nc = _get_program()
    res = run_bass_kernel_spmd(nc, in_maps, list(range(N_CORES)))
    return assemble_output(res.results)


# revision 3
# speedup vs baseline: 1.0928x; 1.0928x over previous
"""Trainium2 Bass kernel for the stacked-LSTMCell network.

Semantics recap (derived from the reference):
  - enc [T=256, B=2048, 136] is flat-reinterpreted so that the scan runs
    2048 independent sequences of length 256.  Flat row r = t*2048 + b
    belongs to sequence s' = r // 256 at step j = r % 256.
  - Sharding: core m owns t in [32m, 32m+32) (=> sequences [256m, 256m+256)).
  - Per core: 256 sequences, 256 steps, feature-major tiles [feat, batch].

Tricks baked into host-side weight preprocessing:
  - tanh(x) = 2*sigmoid(2x) - 1: all gate activations become Sigmoid (one
    ACT table set, one big sigmoid per gate block); the *2 on g-gate inputs
    is folded into the weights, the affine fixups into fused DVE
    scalar_tensor_tensor ops.
  - The h-states are stored halved (H = h/2, exactly absorbing the 2s-1
    fixup); consumers' weights are pre-doubled.
  - LSTM1 bias rides an extra all-ones row of X; LSTM2 bias rides an
    all-ones row appended to the H2 state tile.
"""

import sys

sys.path.insert(0, "/opt/trn_rl_repo")

import numpy as np
import ml_dtypes

import concourse.bass as bass
import concourse.tile as tile
import concourse.mybir as mybir
from concourse import bacc
from concourse.bass_utils import run_bass_kernel_spmd

BF16 = mybir.dt.bfloat16
F32 = mybir.dt.float32
AF = mybir.ActivationFunctionType
OP = mybir.AluOpType
NPBF16 = ml_dtypes.bfloat16

T_FULL, B_FULL = 256, 2048
N_CORES = 8
SEQ = 256            # sequences per core
H1, H2 = 128, 32
EMB_W = [32, 16, 16, 8, 8, 8]


# ---------------------------------------------------------------- weights

def prep_weights(W_ih1, b_ih1, W_hh1, b_hh1, W_ih2, b_ih2, W_hh2, b_hh2,
                 W1, b1, W2, b2):
    """All the prescaling tricks, host side (cheap, <1MB of data)."""
    f32 = np.float32
    W_ih1, b_ih1 = np.asarray(W_ih1, f32), np.asarray(b_ih1, f32)
    W_hh1, b_hh1 = np.asarray(W_hh1, f32), np.asarray(b_hh1, f32)
    W_ih2, b_ih2 = np.asarray(W_ih2, f32), np.asarray(b_ih2, f32)
    W_hh2, b_hh2 = np.asarray(W_hh2, f32), np.asarray(b_hh2, f32)
    W1, b1 = np.asarray(W1, f32), np.asarray(b1, f32)
    W2, b2 = np.asarray(W2, f32), np.asarray(b2, f32)

    g = slice(256, 384)          # g-gate block in the 512-dim (i,f,g,o)
    # LSTM1: x-side weights augmented with a bias row (X row 136 == 1.0).
    wx = np.zeros((144, 512), f32)
    wx[0:136, :] = W_ih1.T
    wx[136, :] = b_ih1 + b_hh1
    wx[:, g] *= 2.0                          # g-gate: feed 2*pre (tanh trick)
    wx_hi = wx[0:128].astype(NPBF16)         # [128, 512]
    wx_lo = wx[128:144].astype(NPBF16)       # [16, 512]

    wh1 = (2.0 * W_hh1).T.copy()             # h1 stored halved
    wh1[:, g] *= 2.0
    wh1 = wh1.astype(NPBF16)                 # [128, 512]

    g2 = slice(64, 96)
    wih2 = (2.0 * W_ih2).T.copy()            # [128, 128]
    wih2[:, g2] *= 2.0
    wih2 = wih2.astype(NPBF16)

    whh2 = np.zeros((33, 128), f32)
    whh2[0:32] = (2.0 * W_hh2).T             # h2 stored halved
    whh2[32] = b_ih2 + b_hh2                 # bias row (H2 row 32 == 1.0)
    whh2[:, g2] *= 2.0
    whh2 = whh2.astype(NPBF16)               # [33, 128]

    w1rep = np.zeros((128, 32), f32)         # 4 stacked copies of (2*W1)^T
    w2blk = np.zeros((128, 4), f32)
    b1rep = np.zeros((128, 1), f32)
    for blk in range(4):
        w1rep[32 * blk:32 * blk + 32, 0:16] = (2.0 * W1).T
        w2blk[32 * blk:32 * blk + 16, blk] = W2[0]
        b1rep[32 * blk:32 * blk + 16, 0] = b1
    return dict(
        wx_hi=wx_hi, wx_lo=wx_lo, wh1=wh1, wih2=wih2, whh2=whh2,
        w1rep=w1rep.astype(NPBF16), w2blk=w2blk.astype(NPBF16),
        b1rep=b1rep, b2=float(b2[0]),
    )


# ------------------------------------------------------------- host X^T

def host_build_xt(mlp_tmpl_c, mlp_static_c, idx_c, tables_f32):
    """Per-core X^T (feature-major, (j,s)-column-ordered), bf16.

    mlp_tmpl_c [32,2048,32], mlp_static_c [32,2048,16], idx_c [32,2048,6].
    Returns xt_hi [128, 65536], xt_lo [16, 65536].
    """
    rows = mlp_tmpl_c.shape[0] * 2048
    X = np.zeros((rows, 144), np.float32)
    X[:, 0:32] = mlp_tmpl_c.reshape(rows, 32)
    X[:, 32:48] = mlp_static_c.reshape(rows, 16)
    idx = idx_c.reshape(rows, 6)
    off = 48
    for c, tbl in enumerate(tables_f32):
        w = tbl.shape[1]
        X[:, off:off + w] = tbl[idx[:, c]]
        off += w
    X[:, 136] = 1.0
    # rows r = 256*s + j  ->  cols c = j*256 + s
    Xp = X.reshape(256, 256, 144).transpose(1, 0, 2).reshape(rows, 144)
    XpT = np.ascontiguousarray(Xp.T).astype(NPBF16)
    return XpT[0:128], XpT[128:144]


# ------------------------------------------------------------ the program

def build_program(J=256, G=2, CHUNK=16):
    """Build + finalize the per-core Bass program.  J = steps to run."""
    SG = SEQ // G
    nc = bacc.Bacc()

    xt_hi = nc.declare_dram_parameter("xt_hi", [128, SEQ * 256], BF16, isOutput=False)
    xt_lo = nc.declare_dram_parameter("xt_lo", [16, SEQ * 256], BF16, isOutput=False)
    wxh = nc.declare_dram_parameter("wx_hi", [128, 512], BF16, isOutput=False)
    wxl = nc.declare_dram_parameter("wx_lo", [16, 512], BF16, isOutput=False)
    wh1d = nc.declare_dram_parameter("wh1", [128, 512], BF16, isOutput=False)
    wih2d = nc.declare_dram_parameter("wih2", [128, 128], BF16, isOutput=False)
    whh2d = nc.declare_dram_parameter("whh2", [33, 128], BF16, isOutput=False)
    w1d = nc.declare_dram_parameter("w1rep", [128, 32], BF16, isOutput=False)
    w2d = nc.declare_dram_parameter("w2blk", [128, 4], BF16, isOutput=False)
    b1d = nc.declare_dram_parameter("b1rep", [128, 1], F32, isOutput=False)
    b2_host = nc.declare_dram_parameter("b2h", [128, 1], F32, isOutput=False)
    out_d = nc.declare_dram_parameter("out", [256, 256], F32, isOutput=True)

    h2_dram = nc.dram_tensor("h2buf", [256, 32, 256], BF16)

    assert J % 4 == 0
    n_chunks = (J + CHUNK - 1) // CHUNK

    with tile.TileContext(nc) as tc:
        with (
            tc.tile_pool(name="wpool", bufs=1) as wpool,
            tc.tile_pool(name="state", bufs=1) as state,
            tc.tile_pool(name="xstage", bufs=2) as xstage,
            tc.tile_pool(name="work1", bufs=3) as work1,
            tc.tile_pool(name="work2", bufs=2) as work2,
            tc.tile_pool(name="g1p", bufs=2, space="PSUM") as g1p,
            tc.tile_pool(name="g2p", bufs=2, space="PSUM") as g2p,
        ):
            # ---- load weights
            wx_hi_t = wpool.tile([128, 512], BF16)
            nc.sync.dma_start(wx_hi_t[:], wxh[:])
            wx_lo_t = wpool.tile([16, 512], BF16)
            nc.sync.dma_start(wx_lo_t[:], wxl[:])
            wh1_t = wpool.tile([128, 512], BF16)
            nc.sync.dma_start(wh1_t[:], wh1d[:])
            wih2_t = wpool.tile([128, 128], BF16)
            nc.sync.dma_start(wih2_t[:], wih2d[:])
            whh2_t = wpool.tile([33, 128], BF16)
            nc.sync.dma_start(whh2_t[:], whh2d[:])
            # ---- persistent state
            c1 = state.tile([128, 256], F32, tag="c1")
            nc.vector.memset(c1[:], 0.0)
            H1s = []
            for k in range(2):
                t = state.tile([128, 256], BF16, tag=f"H1_{k}")
                nc.vector.memset(t[:], 0.0)
                H1s.append(t)
            H2s = []
            for k in range(2):
                t = state.tile([33, 256], BF16, tag=f"H2_{k}")
                nc.vector.memset(t[0:32, :], 0.0)
                nc.vector.memset(t[32:33, :], 1.0)
                H2s.append(t)
            c2 = state.tile([32, 256], F32, tag="c2")
            nc.vector.memset(c2[:], 0.0)

            # ---- staged X^T chunks (prefetched one chunk ahead)
            xhi_tiles, xlo_tiles = {}, {}

            def stage(ch):
                cols = min(CHUNK, J - ch * CHUNK) * 256
                xhi = xstage.tile([128, CHUNK * 256], BF16, tag="xhi")
                nc.sync.dma_start(
                    xhi[:, 0:cols],
                    xt_hi[:, ch * CHUNK * 256: ch * CHUNK * 256 + cols])
                xlo = xstage.tile([16, CHUNK * 256], BF16, tag="xlo")
                nc.sync.dma_start(
                    xlo[:, 0:cols],
                    xt_lo[:, ch * CHUNK * 256: ch * CHUNK * 256 + cols])
                xhi_tiles[ch], xlo_tiles[ch] = xhi, xlo

            def x_mms(g1t, j):
                ch, jj = divmod(j, CHUNK)
                xc = jj * 256
                for gate in range(4):
                    o = g1t[:, gate * 256:(gate + 1) * 256]
                    nc.tensor.matmul(
                        o, wx_hi_t[:, gate * 128:(gate + 1) * 128],
                        xhi_tiles[ch][:, xc:xc + 256], start=True, stop=False)
                    nc.tensor.matmul(
                        o, wx_lo_t[:, gate * 128:(gate + 1) * 128],
                        xlo_tiles[ch][:, xc:xc + 256], start=False, stop=False)

            stage(0)
            g1_cur = g1p.tile([128, 1024], F32, tag="g1")
            x_mms(g1_cur, 0)

            # ---- recurrent loop
            for j in range(J):
                ch, jj = divmod(j, CHUNK)
                if jj == 0 and ch + 1 < n_chunks:
                    stage(ch + 1)
                H1prev, H1cur = H1s[j % 2], H1s[(j + 1) % 2]
                H2prev, H2cur = H2s[j % 2], H2s[(j + 1) % 2]
                # recurrent h-part lands on the prefilled x-part
                for gate in range(4):
                    nc.tensor.matmul(
                        g1_cur[:, gate * 256:(gate + 1) * 256],
                        wh1_t[:, gate * 128:(gate + 1) * 128],
                        H1prev[:], start=False, stop=True)
                # prefill next step's x-part so the PE queue stays dense
                if j + 1 < J:
                    g1_next = g1p.tile([128, 1024], F32, tag="g1")
                    x_mms(g1_next, j + 1)
                S1 = work1.tile([128, 1024], BF16, tag="S1")
                nc.scalar.activation(S1[:], g1_cur[:], AF.Sigmoid)
                u = work1.tile([128, 256], BF16, tag="u")
                nc.vector.scalar_tensor_tensor(
                    u[:], S1[:, 512:768], 0.5, S1[:, 0:256],
                    OP.subtract, OP.mult)
                v = work1.tile([128, 256], F32, tag="v")
                nc.vector.tensor_tensor(
                    v[:], S1[:, 256:512], c1[:], op=OP.mult)
                nc.vector.scalar_tensor_tensor(
                    c1[:], u[:], 2.0, v[:], OP.mult, OP.add)
                Sc = work1.tile([128, 256], BF16, tag="Sc")
                nc.scalar.activation(Sc[:], c1[:], AF.Sigmoid, scale=2.0)
                nc.vector.scalar_tensor_tensor(
                    H1cur[:], Sc[:], 0.5, S1[:, 768:1024],
                    OP.subtract, OP.mult)
                # ---------------- LSTM2 (whole 256 batch)
                g2 = g2p.tile([32, 1024], F32, tag="g2")
                for gate in range(4):
                    o = g2[:, gate * 256:(gate + 1) * 256]
                    nc.tensor.matmul(
                        o, wih2_t[:, 32 * gate:32 * gate + 32],
                        H1cur[:], start=True, stop=False)
                    nc.tensor.matmul(
                        o, whh2_t[:, 32 * gate:32 * gate + 32],
                        H2prev[:], start=False, stop=True)
                S2 = work2.tile([32, 1024], BF16, tag="S2")
                nc.scalar.activation(S2[:], g2[:], AF.Sigmoid)
                u2 = work2.tile([32, 256], BF16, tag="u2")
                nc.vector.scalar_tensor_tensor(
                    u2[:], S2[:, 512:768], 0.5, S2[:, 0:256],
                    OP.subtract, OP.mult)
                v2 = work2.tile([32, 256], F32, tag="v2")
                nc.vector.tensor_tensor(
                    v2[:], S2[:, 256:512], c2[:], op=OP.mult)
                nc.vector.scalar_tensor_tensor(
                    c2[:], u2[:], 2.0, v2[:], OP.mult, OP.add)
                Sc2 = work2.tile([32, 256], BF16, tag="Sc2")
                nc.scalar.activation(Sc2[:], c2[:], AF.Sigmoid, scale=2.0)
                nc.vector.scalar_tensor_tensor(
                    H2cur[0:32, :], Sc2[:], 0.5, S2[:, 768:1024],
                    OP.subtract, OP.mult)
                nc.sync.dma_start(h2_dram[j], H2cur[0:32, :])
                if j + 1 < J:
                    g1_cur = g1_next

        # ---------------- output head (no recurrence -> bulk)
        with (
            tc.tile_pool(name="owpool", bufs=1) as owpool,
            tc.tile_pool(name="opool", bufs=2) as opool,
            tc.tile_pool(name="olin", bufs=2, space="PSUM") as olin,
            tc.tile_pool(name="oout", bufs=2, space="PSUM") as oout,
        ):
            w1rep_t = owpool.tile([128, 32], BF16)
            nc.sync.dma_start(w1rep_t[:], w1d[:])
            w2blk_t = owpool.tile([128, 4], BF16)
            nc.sync.dma_start(w2blk_t[:], w2d[:])
            b1rep_t = owpool.tile([128, 1], F32)
            nc.sync.dma_start(b1rep_t[:], b1d[:])
            b2t2 = owpool.tile([128, 1], F32)
            nc.sync.dma_start(b2t2[:], b2_host[:])

            out_ps = None
            for q in range(J // 4):
                h2t = opool.tile([128, 256], BF16, tag="h2t")
                nc.sync.dma_start(h2t[:], h2_dram[4 * q:4 * q + 4])
                lin_ps = olin.tile([128, 256], F32, tag="lin")
                for blk in range(4):
                    nc.tensor.matmul(
                        lin_ps[32 * blk:32 * blk + 32, :],
                        w1rep_t[32 * blk:32 * blk + 32, :],
                        h2t[32 * blk:32 * blk + 32, :],
                        start=True, stop=True,
                        tile_position=(32 * blk, 32 * blk))
                lin_sb = opool.tile([128, 256], BF16, tag="linsb")
                nc.scalar.activation(
                    lin_sb[:], lin_ps[:], AF.Relu, bias=b1rep_t[:, 0:1])
                if q % 4 == 0:
                    out_ps = oout.tile([128, 256], F32, tag="outps")
                qi = q % 4
                nc.tensor.matmul(
                    out_ps[32 * qi:32 * qi + 4, :], w2blk_t[:], lin_sb[:],
                    start=True, stop=True, tile_position=(0, 32 * qi))
                if qi == 3 or q == J // 4 - 1:
                    out_sb = opool.tile([128, 256], F32, tag="outsb")
                    nc.scalar.activation(
                        out_sb[:], out_ps[:], AF.Sigmoid, bias=b2t2[:, 0:1])
                    Q = q // 4
                    for k in range(qi + 1):
                        nc.sync.dma_start(
                            out_d[16 * Q + 4 * k:16 * Q + 4 * k + 4, :],
                            out_sb[32 * k:32 * k + 4, :])

    nc.finalize()
    return nc


# ------------------------------------------------------------------ entry

_PROGRAM_CACHE = {}


def _get_program(J=256, G=2, CHUNK=16):
    key = (J, G, CHUNK)
    if key not in _PROGRAM_CACHE:
        _PROGRAM_CACHE[key] = build_program(J, G, CHUNK)
    return _PROGRAM_CACHE[key]


def make_in_maps(mlp_static_data, mlp_tmpl_data, emb_static_data,
                 emb_tmpl_data, emb_tables, weights):
    """Shard + host-prep the full inputs into 8 per-core input maps."""
    tables_f32 = [np.asarray(t, np.float32) for t in emb_tables]
    idx_full = np.concatenate(
        [np.asarray(emb_tmpl_data), np.asarray(emb_static_data)],
        axis=2).astype(np.int32)
    mlp_tmpl = np.asarray(mlp_tmpl_data, np.float32)
    mlp_static = np.asarray(mlp_static_data, np.float32)
    in_maps = []
    for m in range(N_CORES):
        sl = slice(32 * m, 32 * m + 32)
        xt_hi, xt_lo = host_build_xt(
            mlp_tmpl[sl], mlp_static[sl], idx_full[sl], tables_f32)
        in_maps.append(dict(
            xt_hi=xt_hi, xt_lo=xt_lo,
            wx_hi=weights["wx_hi"], wx_lo=weights["wx_lo"],
            wh1=weights["wh1"], wih2=weights["wih2"], whh2=weights["whh2"],
            w1rep=weights["w1rep"], w2blk=weights["w2blk"],
            b1rep=weights["b1rep"],
            b2h=np.full((128, 1), weights["b2"], np.float32),
        ))
    return in_maps


def assemble_output(results):
    outs = []
    for m in range(N_CORES):
        od = np.asarray(results[m]["out"], np.float32)   # [j, s]
        outs.append(od.T.reshape(32, 2048))              # r = 256 s + j
    return np.concatenate(outs, axis=0)


def kernel(mlp_static_data, mlp_tmpl_data, emb_static_data, emb_tmpl_data,
           emb_tables, W_ih1, b_ih1, W_hh1, b_hh1, W_ih2, b_ih2, W_hh2,
           b_hh2, W1, b1, W2, b2):
    weights = prep_weights(W_ih1, b_ih1, W_hh1, b_hh1, W_ih2, b_ih2,
                           W_hh2, b_hh2, W1, b1, W2, b2)
    in_maps = make_in_maps(mlp_static_data, mlp_tmpl_data, emb_static_data,
                           emb_tmpl_data, emb_tables, weights)
    nc = _get_program()
    res = run_bass_kernel_spmd(nc, in_maps, list(range(N_CORES)))
    return assemble_output(res.results)
